# revision 1
# baseline (speedup 1.0000x reference)
"""MFA block kernel for 8 Trainium2 NeuronCores.

Full (unsharded) inputs in, full output out. Tokens (8*1024 = 8192) are
sharded across 8 cores (1024 each).  Uses the associative rewrite

    y = theta_x @ (phi_x^T @ g_x) / BN

so the (BN, BN) attention matrix is never formed.  With X_ext = [x_l | 1],

    M = phi_x^T g_x = P_ext^T (X_ext^T X_ext) G_ext,  P_ext = [phi_w; phi_b],
                                                      G_ext = [g_w; g_b],

so only C_ext = X_ext^T X_ext (257x257) needs an AllReduce.  C_ext is
symmetric, so the payload is triangle-packed as two rectangles (rows 0:128
x all cols, rows 128:256 x cols 128:257); the mirrored block and the s-row
are read back transposed directly from the DRAM bounce buffer.

BatchNorm (training mode) needs global per-feature sum / sum-of-squares of
w_y' = theta_x @ (M/BN) @ w_w  -- a second, tiny (2x512) AllReduce.  w_b is
dropped entirely: BN output is invariant to constant input shifts.

Everything on the x_h side lives feature-major ([feature, token]) so that
BN stats are free-dim reduces and the BN apply is a per-partition
tensor_scalar; x_h^T is loaded and z^T stored via transposed-AP DMAs.
"""

import threading

import numpy as np

import concourse.tile as tile
from concourse import bacc, mybir
from concourse.bass_utils import run_bass_kernel_spmd

FP = mybir.dt.float32
HIGH = 512
LOW = 256
B = 8
N = 1024
BN = B * N            # 8192 flattened tokens
NCORES = 8
TPC = BN // NCORES    # 1024 tokens per core
TT = TPC // 128       # 8 token tiles per core
HC = HIGH // 128      # 4 feature chunks of x_h / w_y / z
EPS = 1e-5

LOWE = LOW + 1        # 257: homogeneous low dim


def build_kernel(repeats: int = 1, noar: bool = False):
    nc = bacc.Bacc("TRN2", target_bir_lowering=False, debug=False,
                   num_devices=NCORES)

    x_h = nc.declare_dram_parameter("x_h", [TPC, HIGH], FP, isOutput=False)
    x_l = nc.declare_dram_parameter("x_l", [TPC, LOW], FP, isOutput=False)
    g_w = nc.declare_dram_parameter("g_w", [LOW, LOW], FP, isOutput=False)
    g_b = nc.declare_dram_parameter("g_b", [LOW], FP, isOutput=False)
    theta_w = nc.declare_dram_parameter("theta_w", [HIGH, LOW], FP, isOutput=False)
    theta_b = nc.declare_dram_parameter("theta_b", [LOW], FP, isOutput=False)
    phi_w = nc.declare_dram_parameter("phi_w", [LOW, LOW], FP, isOutput=False)
    phi_b = nc.declare_dram_parameter("phi_b", [LOW], FP, isOutput=False)
    w_w = nc.declare_dram_parameter("w_w", [LOW, HIGH], FP, isOutput=False)
    bn_gamma = nc.declare_dram_parameter("bn_gamma", [HIGH], FP, isOutput=False)
    bn_beta = nc.declare_dram_parameter("bn_beta", [HIGH], FP, isOutput=False)
    z_out = nc.declare_dram_parameter("z", [TPC, HIGH], FP, isOutput=True)

    rg = [list(range(NCORES))]

    with tile.TileContext(nc) as tc:
        with (
            tc.tile_pool(name="sb", bufs=1) as sb,
            tc.tile_pool(name="ps", bufs=1, space="PSUM") as ps,
            tc.tile_pool(name="dram", bufs=1, space="DRAM") as dram,
        ):
            # ---- small constants (chain weights load later, after the
            #      input DMAs, so inputs win the DMA queues)
            eps_c = sb.tile([128, 1], FP, tag="eps_c")
            nc.vector.memset(eps_c, EPS)

            for _ in range(repeats):
                # ---- x_l load (token-major) + homogeneous ones column
                xle = sb.tile([128, TT, LOWE], FP, tag="xle")
                nc.vector.memset(xle[:, :, LOW:LOWE], 1.0)
                for i in range(TT):
                    nc.sync.dma_start(xle[:, i, 0:LOW],
                                      x_l[i * 128:(i + 1) * 128, :])

                # ---- x_h^T via transposed-AP DMA loads (feature-major)
                xht = sb.tile([128, HC, TPC], FP, tag="xht")
                with nc.allow_non_contiguous_dma(reason="transposed x_h load"):
                    for hc in range(HC):
                        nc.sync.dma_start(
                            xht[:, hc, :],
                            x_h[:, hc * 128:(hc + 1) * 128].rearrange(
                                "t p -> p t"))

                # ---- weights: thw/thb feed thetaT (runs under AR1);
                #      gext/pext/ww/gamma/beta are only needed post-AR1.
                thw = sb.tile([128, HIGH // 128, LOW], FP, tag="thw")
                nc.sync.dma_start(thw[:], theta_w[:, :].rearrange(
                    "(ko ki) a -> ki ko a", ki=128))
                thb = sb.tile([128, LOW // 128], FP, tag="thb")
                nc.sync.dma_start(thb[:], theta_b[:].rearrange(
                    "(ko ki) -> ki ko", ki=128))
                gext = sb.tile([128, 3, LOW], FP, tag="gext")
                nc.sync.dma_start(gext[:, 0:2, :], g_w[:, :].rearrange(
                    "(ko ki) a -> ki ko a", ki=128))
                nc.sync.dma_start(gext[0:1, 2, :], g_b[:][None, :])
                pext = sb.tile([128, 3, LOW], FP, tag="pext")
                nc.sync.dma_start(pext[:, 0:2, :], phi_w[:, :].rearrange(
                    "(ko ki) a -> ki ko a", ki=128))
                nc.sync.dma_start(pext[0:1, 2, :], phi_b[:][None, :])
                ww = sb.tile([128, LOW // 128, HIGH], FP, tag="ww")
                nc.sync.dma_start(ww[:], w_w[:, :].rearrange(
                    "(ko ki) h -> ki ko h", ki=128))
                gamma_p = sb.tile([128, HC], FP, tag="gamma_p")
                nc.sync.dma_start(gamma_p[:], bn_gamma[:].rearrange(
                    "(hc p) -> p hc", p=128))
                beta_p = sb.tile([128, HC], FP, tag="beta_p")
                nc.sync.dma_start(beta_p[:], bn_beta[:].rearrange(
                    "(hc p) -> p hc", p=128))

                # ---- C_rect = X_ext[:, 0:256]^T @ X_ext  (256 x 257)
                cl = sb.tile([128, 2, LOWE], FP, tag="cl")
                for mc in range(2):
                    cps = ps.tile([128, 512], FP, tag="mm", bufs=4)
                    for i in range(TT):
                        nc.tensor.matmul(
                            cps[:, :LOWE],
                            xle[:, i, mc * 128:(mc + 1) * 128],
                            xle[:, i, :],
                            start=(i == 0), stop=(i == TT - 1))
                    nc.vector.tensor_copy(cl[:, mc, :], cps[:, :LOWE])

                # ---- AllReduce #1: triangle-packed C (198KB)
                # c_in[:, 0:257]  = C rows 0:128, all cols (incl. s col)
                # c_in[:, 257:386] = C rows 128:256, cols 128:257
                c_in = dram.tile([128, 386], FP, tag="c_in")
                c_out = dram.tile([128, 386], FP, tag="c_out")
                nc.sync.dma_start(c_in[:, 0:LOWE], cl[:, 0, :])
                nc.sync.dma_start(c_in[:, LOWE:386], cl[:, 1, 128:LOWE])
                if noar:
                    nc.sync.dma_start(c_out[:, :], c_in[:, :])
                else:
                    nc.gpsimd.collective_compute(
                        "AllReduce", mybir.AluOpType.add, replica_groups=rg,
                        ins=[c_in.opt()], outs=[c_out.opt()])
                # Reconstruct the three k-tiles of C_ext from the bounce:
                # cga = rows 0:128 (all 257 cols);  cgb = rows 128:256:
                #   cols 0:128 mirrored from block(0,1)^T, cols 128:257 direct.
                # srow = s^T (row 256) from the s columns; corner = BN.
                cga = sb.tile([128, LOWE], FP, tag="cga")
                nc.sync.dma_start(cga[:], c_out[:, 0:LOWE])
                cgb = sb.tile([128, LOWE], FP, tag="cgb")
                nc.sync.dma_start(cgb[:, 128:LOWE], c_out[:, LOWE:386])
                srow = sb.tile([1, LOWE], FP, tag="srow")
                with nc.allow_non_contiguous_dma(reason="transposed C read"):
                    nc.sync.dma_start(
                        cgb[:, 0:128],
                        c_out[:, 128:256].rearrange("p q -> q p"))
                    nc.sync.dma_start(
                        srow[:, 0:128],
                        c_out[:, 256:LOWE].rearrange("p o -> o p"))
                    nc.sync.dma_start(
                        srow[:, 128:256],
                        c_out[:, 385:386].rearrange("p o -> o p"))
                nc.vector.memset(srow[:, 256:LOWE], float(BN))

                # ---- thetaT = theta_w^T @ x_h^T + theta_b  (feature-major;
                #      overlaps AR1)
                tht = sb.tile([128, LOW // 128, TPC], FP, tag="tht")
                for mc in range(LOW // 128):
                    for nn in range(TPC // 512):
                        tps = ps.tile([128, 512], FP, tag="mm", bufs=4)
                        for k in range(HIGH // 128):
                            nc.tensor.matmul(
                                tps, thw[:, k, mc * 128:(mc + 1) * 128],
                                xht[:, k, nn * 512:(nn + 1) * 512],
                                start=(k == 0), stop=(k == HIGH // 128 - 1))
                        nc.vector.tensor_scalar(
                            tht[:, mc, nn * 512:(nn + 1) * 512], tps,
                            thb[:, mc:mc + 1], None, mybir.AluOpType.add)

                # ---- T1 = C_ext @ G_ext  (257 x 256)
                cg_tiles = [cga, cgb, srow]
                t1 = sb.tile([128, 3, LOW], FP, tag="t1")
                for mc in range(3):
                    msl = (slice(0, 128), slice(128, 256),
                           slice(256, 257))[mc]
                    mlen = msl.stop - msl.start
                    t1f = ps.tile([128, 512], FP, tag="mm", bufs=4)
                    t1ps = t1f[:mlen, :LOW]
                    for k in range(3):
                        klen = 128 if k < 2 else 1
                        nc.tensor.matmul(t1ps, cg_tiles[k][:klen, msl],
                                         gext[:klen, k, :],
                                         start=(k == 0), stop=(k == 2))
                    nc.vector.tensor_copy(t1[:mlen, mc, :], t1ps)

                # ---- MpT = (T1^T @ P_ext) / BN   (M'^T, 256 x 256)
                mpt = sb.tile([128, LOW // 128, LOW], FP, tag="mpt")
                for bc in range(LOW // 128):
                    mpf = ps.tile([128, 512], FP, tag="mm", bufs=4)
                    mps = mpf[:, :LOW]
                    for k in range(3):
                        klen = 128 if k < 2 else 1
                        nc.tensor.matmul(
                            mps, t1[:klen, k, bc * 128:(bc + 1) * 128],
                            pext[:klen, k, :],
                            start=(k == 0), stop=(k == 2))
                    nc.vector.tensor_scalar_mul(mpt[:, bc, :], mps, 1.0 / BN)

                # ---- V = M' @ w_w   (256 x 512)
                v = sb.tile([128, LOW // 128, HIGH], FP, tag="v")
                for ac in range(LOW // 128):
                    vps = ps.tile([128, 512], FP, tag="mm", bufs=4)
                    for k in range(LOW // 128):
                        nc.tensor.matmul(
                            vps, mpt[:, k, ac * 128:(ac + 1) * 128],
                            ww[:, k, :], start=(k == 0),
                            stop=(k == LOW // 128 - 1))
                    nc.vector.tensor_copy(v[:, ac, :], vps)

                # ---- w_y'^T = V^T-chunks @ thetaT   (feature-major)
                wyt = sb.tile([128, HC, TPC], FP, tag="wyt")
                for hc in range(HC):
                    for nn in range(TPC // 512):
                        wps = ps.tile([128, 512], FP, tag="mm", bufs=4)
                        for k in range(LOW // 128):
                            nc.tensor.matmul(
                                wps, v[:, k, hc * 128:(hc + 1) * 128],
                                tht[:, k, nn * 512:(nn + 1) * 512],
                                start=(k == 0), stop=(k == LOW // 128 - 1))
                        nc.vector.tensor_copy(
                            wyt[:, hc, nn * 512:(nn + 1) * 512], wps)

                # ---- BN stats: per-partition free-dim reduces
                sqt = sb.tile([128, HC, TPC], FP, tag="sqt")
                nc.scalar.activation(sqt[:], wyt[:],
                                     mybir.ActivationFunctionType.Square)
                ssum = sb.tile([128, HC], FP, tag="ssum")
                nc.vector.reduce_sum(ssum[:], wyt[:],
                                     axis=mybir.AxisListType.X)
                ssq = sb.tile([128, HC], FP, tag="ssq")
                nc.vector.reduce_sum(ssq[:], sqt[:],
                                     axis=mybir.AxisListType.X)

                # ---- AllReduce #2 (stats, 4KB)
                s_in = dram.tile([2, HIGH], FP, tag="s_in")
                s_out = dram.tile([2, HIGH], FP, tag="s_out")
                nc.sync.dma_start(
                    s_in[0, :].rearrange("(hc p) -> p hc", p=128), ssum[:])
                nc.sync.dma_start(
                    s_in[1, :].rearrange("(hc p) -> p hc", p=128), ssq[:])
                if noar:
                    nc.sync.dma_start(s_out[:, :], s_in[:, :])
                else:
                    nc.gpsimd.collective_compute(
                        "AllReduce", mybir.AluOpType.add, replica_groups=rg,
                        ins=[s_in.opt()], outs=[s_out.opt()])
                sgs = sb.tile([128, HC], FP, tag="sgs")
                nc.sync.dma_start(
                    sgs[:], s_out[0, :].rearrange("(hc p) -> p hc", p=128))
                sgq = sb.tile([128, HC], FP, tag="sgq")
                nc.sync.dma_start(
                    sgq[:], s_out[1, :].rearrange("(hc p) -> p hc", p=128))

                # ---- r = x_h^T + beta  (independent of AR2 -> overlaps it)
                r_t = sb.tile([128, HC, TPC], FP, tag="r_t")
                for hc in range(HC):
                    nc.vector.tensor_scalar(
                        r_t[:, hc, :], xht[:, hc, :], beta_p[:, hc:hc + 1],
                        None, mybir.AluOpType.add)

                # ---- A = gamma * rsqrt(var+eps); all per-partition [128, HC]
                mean_p = sb.tile([128, HC], FP, tag="mean_p")
                nc.vector.tensor_scalar_mul(mean_p[:], sgs[:], 1.0 / BN)
                ex2_p = sb.tile([128, HC], FP, tag="ex2_p")
                nc.vector.tensor_scalar_mul(ex2_p[:], sgq[:], 1.0 / BN)
                msq_p = sb.tile([128, HC], FP, tag="msq_p")
                nc.vector.tensor_mul(msq_p[:], mean_p[:], mean_p[:])
                var_p = sb.tile([128, HC], FP, tag="var_p")
                nc.vector.tensor_sub(var_p[:], ex2_p[:], msq_p[:])
                std_p = sb.tile([128, HC], FP, tag="std_p")
                nc.scalar.activation(std_p[:], var_p[:],
                                     mybir.ActivationFunctionType.Sqrt,
                                     bias=eps_c[:])
                nc.vector.reciprocal(std_p[:], std_p[:])
                a_p = sb.tile([128, HC], FP, tag="a_p")
                nc.vector.tensor_mul(a_p[:], gamma_p[:], std_p[:])

                # ---- z^T = (w_y'^T - mean)*A + (x_h^T + beta); store
                zt = sb.tile([128, HC, TPC], FP, tag="zt")
                with nc.allow_non_contiguous_dma(reason="transposed z store"):
                    for hc in range(HC):
                        nc.vector.tensor_scalar(
                            zt[:, hc, :], wyt[:, hc, :],
                            mean_p[:, hc:hc + 1], a_p[:, hc:hc + 1],
                            mybir.AluOpType.subtract, mybir.AluOpType.mult)
                        nc.vector.tensor_add(zt[:, hc, :], zt[:, hc, :],
                                             r_t[:, hc, :])
                        nc.sync.dma_start(
                            z_out[:, hc * 128:(hc + 1) * 128].rearrange(
                                "t p -> p t"),
                            zt[:, hc, :])

    nc.compile()
    return nc


_CACHE: dict[int, "bacc.Bacc"] = {}
_LOCK = threading.Lock()


def _get_nc(repeats: int = 1):
    with _LOCK:
        if repeats not in _CACHE:
            _CACHE[repeats] = build_kernel(repeats)
        return _CACHE[repeats]


def _shard_inputs(inputs: dict) -> list[dict]:
    xh = np.ascontiguousarray(
        np.asarray(inputs["x_h"], dtype=np.float32).reshape(BN, HIGH))
    xl = np.ascontiguousarray(
        np.asarray(inputs["x_l"], dtype=np.float32).reshape(BN, LOW))
    # w_b is intentionally unused: BatchNorm output is invariant to a
    # constant shift of its input, so the w_b add cancels exactly.
    common = {
        "g_w": np.asarray(inputs["g_w"], np.float32),
        "g_b": np.asarray(inputs["g_b"], np.float32),
        "theta_w": np.asarray(inputs["theta_w"], np.float32),
        "theta_b": np.asarray(inputs["theta_b"], np.float32),
        "phi_w": np.asarray(inputs["phi_w"], np.float32),
        "phi_b": np.asarray(inputs["phi_b"], np.float32),
        "w_w": np.asarray(inputs["w_w"], np.float32),
        "bn_gamma": np.asarray(inputs["bn_gamma"], np.float32),
        "bn_beta": np.asarray(inputs["bn_beta"], np.float32),
    }
    return [
        {"x_h": xh[c * TPC:(c + 1) * TPC],
         "x_l": xl[c * TPC:(c + 1) * TPC], **common}
        for c in range(NCORES)
    ]


def kernel(**inputs) -> np.ndarray:
    nc = _get_nc(1)
    in_maps = _shard_inputs(inputs)
    res = run_bass_kernel_spmd(nc, in_maps, list(range(NCORES)))
    z = np.concatenate([res.results[c]["z"] for c in range(NCORES)], axis=0)
    return z.reshape(B, N, HIGH)

